# revision 1
# baseline (speedup 1.0000x reference)
"""Trainium2 Bass kernel for nn_DecoderTransformer (T=2048, D=2048, H=16, V=32000).

Strategy (8-way tensor parallel, full inputs in / full output out):
  - Each core computes full x = we[tok] + pe via indirect-DMA gather; x is
    transposed on-chip to xT [D, T] (held in SBUF as two t-halves and spilled
    to a DRAM scratch for the residual add later). qT/kT/vT projections are
    interleaved with the gather per 512-column t-block so the PE works under
    the gather's DMA latency.
  - Heads are sharded 2-per-core: causal attention is done blockwise with
    softmax-without-max (sims range is ~[-11, 12], exp is safe in f32),
    producing headsT [2*hs, T].
  - AllGather(headsT) -> catT [D, T]; proj output is sharded over d_out
    (256 rows per core), AllGather -> saT [D, T]. Both AllGathers are split
    into t-halves so they overlap attention/proj/fc compute.
  - resid = x + sa + proj_b built on-chip; fc is sharded over vocab
    (4000 cols per core) producing logitsT [4000, T]; host transposes and
    concatenates the shards.
  - All matmuls run as float32r (full-rate fp32 on the PE, ~2^-13 rounding).
"""

import os

import numpy as np

T = 2048
D = 2048
H = 16
HS = 128
V = 32000
NCORES = 8
P = 128
DC = D // P            # 16 d chunks
TC = T // P            # 16 t chunks
NTB = T // 512         # 4 t-blocks of 512
HPC = H // NCORES      # 2 heads per core
VSH = V // NCORES      # 4000 vocab shard
VCH = 125              # vocab chunk (psum partition dim)
NVC = VSH // VCH       # 32 vocab chunks
DOS = D // NCORES      # 256 d_out shard rows

_CACHE = {}


def _build():
    import concourse.bass as bass
    import concourse.tile as tile
    from concourse import bacc, mybir
    from concourse.masks import make_identity

    f32 = mybir.dt.float32
    f32r = mybir.dt.float32r
    i32 = mybir.dt.int32
    EXP = mybir.ActivationFunctionType.Exp
    RG = [list(range(NCORES))]

    nc = bacc.Bacc("TRN2", target_bir_lowering=False, debug=False,
                   num_devices=NCORES, num_swdge_queues=4)

    tok = nc.dram_tensor("tok", [T], i32, kind="ExternalInput")
    we = nc.dram_tensor("we", [V, D], f32, kind="ExternalInput")
    pe_d = nc.dram_tensor("pe", [T, D], f32, kind="ExternalInput")
    wq = nc.dram_tensor("wq", [D, HPC * HS], f32, kind="ExternalInput")
    wk = nc.dram_tensor("wk", [D, HPC * HS], f32, kind="ExternalInput")
    wv = nc.dram_tensor("wv", [D, HPC * HS], f32, kind="ExternalInput")
    pw = nc.dram_tensor("pw", [D, DOS], f32, kind="ExternalInput")
    pb = nc.dram_tensor("pb", [D, 1], f32, kind="ExternalInput")
    fw = nc.dram_tensor("fw", [D, VSH], f32, kind="ExternalInput")
    fb = nc.dram_tensor("fb", [VSH, 1], f32, kind="ExternalInput")
    outT = nc.dram_tensor("outT", [VSH, T], f32, kind="ExternalOutput")

    with tile.TileContext(nc) as tc:
        dram = tc.alloc_tile_pool(name="dram", bufs=1, space="DRAM")
        pconst = tc.alloc_tile_pool(name="pconst", bufs=1)

        ident = pconst.tile([P, P], f32, name="ident")
        make_identity(nc, ident[:])
        ones_f = pconst.tile([P, 1], f32, name="ones_f")
        nc.vector.memset(ones_f[:], 1.0)
        ones_col = pconst.tile([P, 1], f32r, name="ones_col")
        nc.vector.tensor_copy(ones_col[:], ones_f[:])
        ones_row = pconst.tile([1, P], f32, name="ones_row")
        nc.vector.memset(ones_row[:], 1.0)
        pb_s = pconst.tile([P, DC], f32, name="pb_s")
        nc.sync.dma_start(out=pb_s[:],
                          in_=pb[:].rearrange("(dc p) one -> p (dc one)", p=P))
        fb_s = pconst.tile([VCH, NVC], f32, name="fb_s")
        nc.sync.dma_start(out=fb_s[:],
                          in_=fb[:].rearrange("(vc p) one -> p (vc one)", p=VCH))
        # additive causal mask, shifted views: maskadd_m[s, t] with
        # m in 0..3 = maskbig[:, 384-128m : 896-128m]; 0 iff s <= t - 128*m.
        maskbig = pconst.tile([P, 896], f32, name="maskbig")
        nc.gpsimd.memset(maskbig[:], 0.0)
        nc.gpsimd.affine_select(
            out=maskbig[:], in_=maskbig[:],
            compare_op=mybir.AluOpType.is_ge, fill=-40.0,
            base=-384, pattern=[[1, 896]], channel_multiplier=-1,
        )

        xT_dram = dram.tile([D, T], f32, name="xT_dram")
        ag1_in = [dram.tile([HPC * HS, 1024], f32, name=f"ag1_in{h}")
                  for h in range(2)]
        ag1_out = [dram.tile([D, 1024], f32, name=f"ag1_out{h}",
                             addr_space="Shared") for h in range(2)]
        ag2_in = [dram.tile([DOS, 1024], f32, name=f"ag2_in{h}")
                  for h in range(2)]
        ag2_out = [dram.tile([D, 1024], f32, name=f"ag2_out{h}",
                             addr_space="Shared") for h in range(2)]

        with tc.tile_pool(name="pqa", bufs=1) as pqa:
            qT = [pqa.tile([P, T], f32r, name=f"qT{h}") for h in range(HPC)]
            kT = [pqa.tile([P, T], f32r, name=f"kT{h}") for h in range(HPC)]
            vT = [pqa.tile([P, T], f32r, name=f"vT{h}") for h in range(HPC)]
            projs = ((wq, qT), (wk, kT), (wv, vT))

            # ---- Phase 1+2 interleaved: gather/transpose + qkv per t-block
            with tc.tile_pool(name="px", bufs=1) as px, \
                 tc.tile_pool(name="pemb", bufs=1) as pemb, \
                 tc.tile_pool(name="ps_tr", bufs=2, space="PSUM") as ps_tr, \
                 tc.tile_pool(name="ps_qkv", bufs=6, space="PSUM") as psq:
                xTh = [None, None]
                for b in range(NTB):
                    half, sub = b // 2, b % 2
                    if sub == 0:
                        xTh[half] = px.tile([P, DC, 1024], f32r, tag="xTh",
                                            name=f"xTh{half}")
                    for k in range(4):
                        tcc = b * 4 + k
                        idx_t = pemb.tile([P, 1], i32, tag="idx", bufs=3,
                                          name=f"idx{tcc}")
                        nc.sync.dma_start(
                            out=idx_t[:],
                            in_=tok[:][tcc * P:(tcc + 1) * P, None])
                        xg = pemb.tile([P, D], f32, tag="xg", bufs=4,
                                       name=f"xg{tcc}")
                        gi = nc.gpsimd.indirect_dma_start(
                            out=xg[:], out_offset=None, in_=we[:],
                            in_offset=bass.IndirectOffsetOnAxis(
                                ap=idx_t[:, :1], axis=0))
                        if tcc % 4:
                            gi.ins.queue = f"qPoolDynamic{tcc % 4}"
                        pet = pemb.tile([P, D], f32, tag="pet", bufs=3,
                                        name=f"pet{tcc}")
                        nc.scalar.dma_start(
                            out=pet[:], in_=pe_d[:][tcc * P:(tcc + 1) * P, :])
                        nc.vector.tensor_add(xg[:], xg[:], pet[:])
                        for q4 in range(4):
                            tr_ps = ps_tr.tile([P, 512], f32, tag="tr",
                                               name=f"tr{tcc}_{q4}")
                            for k4 in range(4):
                                dc = q4 * 4 + k4
                                nc.tensor.transpose(
                                    tr_ps[:, k4 * P:(k4 + 1) * P],
                                    xg[:, dc * P:(dc + 1) * P], ident[:])
                            nc.vector.tensor_copy(
                                xTh[half][:, q4 * 4:(q4 + 1) * 4,
                                          sub * 512 + k * P:
                                          sub * 512 + (k + 1) * P],
                                tr_ps[:].rearrange("p (c t) -> p c t", c=4))
                        nc.scalar.dma_start(
                            out=xT_dram[:, tcc * P:(tcc + 1) * P].rearrange(
                                "(dc p) t -> p dc t", p=P),
                            in_=xTh[half][:, :, sub * 512 + k * P:
                                          sub * 512 + (k + 1) * P]
                            .bitcast(f32))
                    # qkv projections for this t-block
                    for wdram, outs in projs:
                        for h in range(HPC):
                            w_t = pemb.tile([P, DC, HS], f32r, tag="wqk",
                                            bufs=2, name=f"w{wdram.name}{h}{b}")
                            nc.sync.dma_start(
                                out=w_t[:],
                                in_=wdram[:][:, h * HS:(h + 1) * HS]
                                .bitcast(f32r)
                                .rearrange("(dc p) j -> p dc j", p=P))
                            ps = psq.tile([P, 512], f32, tag="qkps",
                                          name=f"ps{wdram.name}{h}_{b}")
                            for dc in range(DC):
                                nc.tensor.matmul(
                                    ps[:], w_t[:, dc, :],
                                    xTh[half][:, dc, sub * 512:(sub + 1) * 512],
                                    start=(dc == 0), stop=(dc == DC - 1))
                            nc.vector.tensor_copy(
                                outs[h][:, b * 512:(b + 1) * 512], ps[:])

            # ---- Phase 3: causal attention (g outer, head inner) ----
            with tc.tile_pool(name="patt", bufs=1) as patt, \
                 tc.tile_pool(name="ps_sm", bufs=3, space="PSUM") as ps_sm, \
                 tc.tile_pool(name="ps_cs", bufs=2, space="PSUM") as ps_cs, \
                 tc.tile_pool(name="ps_av", bufs=2, space="PSUM") as ps_av, \
                 tc.tile_pool(name="ps_bc", bufs=1, space="PSUM") as ps_bc:
                v_both = patt.tile([P, TC, HPC * HS], f32r, name="v_both")
                for g in range(NTB):
                    t4 = g
                    for h in range(HPC):
                        vt_ps = ps_sm.tile([P, 512], f32, tag="sims",
                                           name=f"vtr{h}_{t4}")
                        for k4 in range(4):
                            tcc = t4 * 4 + k4
                            nc.tensor.transpose(
                                vt_ps[:, k4 * P:(k4 + 1) * P],
                                vT[h][:, tcc * P:(tcc + 1) * P].bitcast(f32),
                                ident[:])
                        nc.vector.tensor_copy(
                            v_both[:, t4 * 4:(t4 + 1) * 4,
                                   h * HS:(h + 1) * HS],
                            vt_ps[:].rearrange("p (c t) -> p c t", c=4))
                    for h in range(HPC):
                        nsc = 4 * g + 4
                        expT = patt.tile([P, TC, 512], f32r, tag="expT",
                                         name=f"expT{h}_{g}")
                        cs_ps = ps_cs.tile([1, 512], f32, tag="cs",
                                           name=f"cs{h}_{g}")
                        for c in range(nsc):
                            s_ps = ps_sm.tile([P, 512], f32, tag="sims",
                                              name=f"sims{h}_{g}_{c}")
                            nc.tensor.matmul(
                                s_ps[:], kT[h][:, c * P:(c + 1) * P],
                                qT[h][:, g * 512:(g + 1) * 512],
                                start=True, stop=True)
                            if c >= 4 * g:
                                m = c - 4 * g
                                nc.vector.tensor_add(
                                    s_ps[:], s_ps[:],
                                    maskbig[:, 384 - 128 * m:896 - 128 * m])
                            nc.scalar.activation(out=expT[:, c, :],
                                                 in_=s_ps[:], func=EXP)
                        for c in range(nsc):
                            nc.tensor.matmul(cs_ps[:], ones_col[:],
                                             expT[:, c, :],
                                             start=(c == 0),
                                             stop=(c == nsc - 1))
                        av_ps = ps_av.tile([P, 512], f32, tag="av",
                                           name=f"av{h}_{g}")
                        for c in range(nsc):
                            nc.tensor.matmul(
                                av_ps[:], v_both[:, c, h * HS:(h + 1) * HS],
                                expT[:, c, :],
                                start=(c == 0), stop=(c == nsc - 1))
                        recip = patt.tile([1, 512], f32, tag="recip",
                                          bufs=2, name=f"rc{h}_{g}")
                        nc.vector.reciprocal(recip[:], cs_ps[:])
                        bc_ps = ps_bc.tile([P, 512], f32, tag="bc",
                                           name=f"bc{h}_{g}")
                        nc.tensor.matmul(bc_ps[:], ones_row[:], recip[:],
                                         start=True, stop=True)
                        bc_s = patt.tile([P, 512], f32, tag="bc_s",
                                         bufs=2, name=f"bcs{h}_{g}")
                        nc.vector.tensor_copy(bc_s[:], bc_ps[:])
                        stage = patt.tile([P, 512], f32, tag="stage",
                                          bufs=2, name=f"st{h}_{g}")
                        nc.vector.tensor_mul(stage[:], av_ps[:], bc_s[:])
                        nc.scalar.dma_start(
                            out=ag1_in[g // 2][h * HS:(h + 1) * HS,
                                               (g % 2) * 512:
                                               (g % 2 + 1) * 512],
                            in_=stage[:])
                    if g % 2 == 1:
                        # ---- AllGather heads for this t-half ----
                        nc.gpsimd.collective_compute(
                            "AllGather", mybir.AluOpType.bypass,
                            replica_groups=RG,
                            ins=[ag1_in[g // 2][:]], outs=[ag1_out[g // 2][:]])

        # ---- Phase 5: proj shard (d_out rows [256i, 256i+256)) ----
        with tc.tile_pool(name="pproj", bufs=1) as ppj, \
             tc.tile_pool(name="ps_pj", bufs=4, space="PSUM") as ps_pj:
            pw_t = ppj.tile([P, DC, DOS], f32r, name="pw_t")
            nc.sync.dma_start(
                out=pw_t[:],
                in_=pw[:].bitcast(f32r).rearrange("(dc p) o -> p dc o", p=P))
            for half in range(2):
                for tb2 in range(2):
                    ps_o = [ps_pj.tile([P, 512], f32, tag="pjps",
                                       name=f"pj{half}{tb2}_{o}")
                            for o in range(2)]
                    for dc in range(DC):
                        cat_t = ppj.tile([P, 512], f32r, tag="catT", bufs=6,
                                         name=f"cat{half}{tb2}_{dc}")
                        eng = nc.sync if dc % 2 == 0 else nc.scalar
                        eng.dma_start(
                            out=cat_t[:],
                            in_=ag1_out[half][dc * P:(dc + 1) * P,
                                              tb2 * 512:(tb2 + 1) * 512]
                            .bitcast(f32r))
                        for o in range(2):
                            nc.tensor.matmul(
                                ps_o[o][:], pw_t[:, dc, o * P:(o + 1) * P],
                                cat_t[:],
                                start=(dc == 0), stop=(dc == DC - 1))
                    for o in range(2):
                        ev = ppj.tile([P, 512], f32, tag="pj_ev", bufs=3,
                                      name=f"pjev{half}{tb2}_{o}")
                        nc.vector.tensor_copy(ev[:], ps_o[o][:])
                        nc.scalar.dma_start(
                            out=ag2_in[half][o * P:(o + 1) * P,
                                             tb2 * 512:(tb2 + 1) * 512],
                            in_=ev[:])
                # ---- AllGather proj shards for this t-half ----
                nc.gpsimd.collective_compute(
                    "AllGather", mybir.AluOpType.bypass, replica_groups=RG,
                    ins=[ag2_in[half][:]], outs=[ag2_out[half][:]])

        # ---- Phase 7: resid = x + sa + pb (per half), then fc shard ----
        with tc.tile_pool(name="pfc", bufs=1) as pfc, \
             tc.tile_pool(name="ps_fc", bufs=8, space="PSUM") as ps_fc:
            residT = [pfc.tile([P, DC, 1024], f32r, name=f"residT{hf}")
                      for hf in range(2)]
            for half in range(2):
                for dc in range(DC):
                    sa_t = pfc.tile([P, 1024], f32, tag="sa_t", bufs=2,
                                    name=f"sa{half}_{dc}")
                    saeng = nc.scalar if dc % 2 == 0 else nc.sync
                    saeng.dma_start(
                        out=sa_t[:],
                        in_=ag2_out[half][dc * P:(dc + 1) * P, :])
                    xd_t = pfc.tile([P, 1024], f32, tag="xd_t", bufs=2,
                                    name=f"xd{half}_{dc}")
                    xdeng = nc.sync if dc % 2 == 0 else nc.scalar
                    xdeng.dma_start(
                        out=xd_t[:],
                        in_=xT_dram[dc * P:(dc + 1) * P,
                                    half * 1024:(half + 1) * 1024])
                    nc.vector.tensor_add(sa_t[:], sa_t[:], xd_t[:])
                    nc.vector.tensor_scalar_add(residT[half][:, dc, :],
                                                sa_t[:], pb_s[:, dc:dc + 1])
            def load_fw(vc, nm):
                t = pfc.tile([P, DC, VCH], f32r, tag="fw_t", bufs=4,
                             name=nm)
                nc.sync.dma_start(
                    out=t[:],
                    in_=fw[:][:, vc * VCH:(vc + 1) * VCH].bitcast(f32r)
                    .rearrange("(dc p) v -> p dc v", p=P))
                return t

            def fc_pass(vc, tbs, fw_t, tag):
                psf = {tb: ps_fc.tile([VCH, 512], f32, tag="fcps",
                                      name=f"fc{tag}_{vc}_{tb}")
                       for tb in tbs}
                for dc in range(DC):
                    for tb in tbs:
                        nc.tensor.matmul(
                            psf[tb][:], fw_t[:, dc, :],
                            residT[tb // 2][:, dc,
                                            (tb % 2) * 512:(tb % 2 + 1) * 512],
                            start=(dc == 0), stop=(dc == DC - 1))
                for tb in tbs:
                    ev = pfc.tile([VCH, 512], f32, tag="fc_ev", bufs=4,
                                  name=f"fcev{tag}_{vc}_{tb}")
                    nc.vector.tensor_scalar_add(ev[:], psf[tb][:],
                                                fb_s[:, vc:vc + 1])
                    nc.scalar.dma_start(
                        out=outT[:][vc * VCH:(vc + 1) * VCH,
                                    tb * 512:(tb + 1) * 512],
                        in_=ev[:])

            # vc 0/1 run their first t-half early (absorbs the AG2b wait),
            # their second half follows; the rest run all four t-blocks.
            fw_cache = {vc: load_fw(vc, f"fwp{vc}") for vc in (0, 1)}
            fc_pass(0, [0, 1], fw_cache[0], "a")
            fc_pass(1, [0, 1], fw_cache[1], "a")
            fc_pass(0, [2, 3], fw_cache[0], "b")
            fc_pass(1, [2, 3], fw_cache[1], "b")
            for vc in range(2, NVC):
                fc_pass(vc, [0, 1, 2, 3], load_fw(vc, f"fw{vc}"), "m")

        dram.release()
        pconst.release()

    nc.compile()
    return nc


def _get_nc():
    if "nc" not in _CACHE:
        _CACHE["nc"] = _build()
    return _CACHE["nc"]


def kernel(token_ids, we, pe, Wq, Wk, Wv, proj_w, proj_b, fc_w, fc_b):
    from concourse.bass_utils import run_bass_kernel_spmd

    tok = np.asarray(token_ids).astype(np.int32)
    we = np.ascontiguousarray(np.asarray(we), dtype=np.float32)
    pe = np.ascontiguousarray(np.asarray(pe), dtype=np.float32)[:T]
    Wq = np.asarray(Wq, dtype=np.float32)
    Wk = np.asarray(Wk, dtype=np.float32)
    Wv = np.asarray(Wv, dtype=np.float32)
    proj_w = np.asarray(proj_w, dtype=np.float32)
    proj_b = np.asarray(proj_b, dtype=np.float32)
    fc_w = np.asarray(fc_w, dtype=np.float32)
    fc_b = np.asarray(fc_b, dtype=np.float32)

    scale = np.float32(1.0 / np.sqrt(HS))
    in_maps = []
    for i in range(NCORES):
        h0 = HPC * i
        wq_i = np.ascontiguousarray(
            np.concatenate([Wq[h0 + j] for j in range(HPC)], axis=1)) * scale
        wk_i = np.ascontiguousarray(
            np.concatenate([Wk[h0 + j] for j in range(HPC)], axis=1))
        wv_i = np.ascontiguousarray(
            np.concatenate([Wv[h0 + j] for j in range(HPC)], axis=1))
        pw_i = np.ascontiguousarray(proj_w[:, DOS * i:DOS * (i + 1)])
        fw_i = np.ascontiguousarray(fc_w[:, VSH * i:VSH * (i + 1)])
        fb_i = np.ascontiguousarray(
            fc_b[VSH * i:VSH * (i + 1)].reshape(VSH, 1))
        in_maps.append({
            "tok": tok, "we": we, "pe": pe,
            "wq": wq_i.astype(np.float32), "wk": wk_i, "wv": wv_i,
            "pw": pw_i, "pb": proj_b.reshape(D, 1),
            "fw": fw_i, "fb": fb_i,
        })

    nc = _get_nc()
    trace = bool(int(os.environ.get("BASSKERNEL_TRACE", "0")))
    res = run_bass_kernel_spmd(nc, in_maps, core_ids=list(range(NCORES)),
                               trace=trace)
    if trace and res.exec_time_ns is not None:
        print(f"HW exec time: {res.exec_time_ns} ns")
        if res.instructions_and_trace is not None:
            print(f"Trace: {res.instructions_and_trace[1]}")

    out = np.empty((T, V), dtype=np.float32)
    for i in range(NCORES):
        out[:, VSH * i:VSH * (i + 1)] = res.results[i]["outT"].T
    return out



# revision 7
# speedup vs baseline: 1.3008x; 1.3008x over previous
"""Trainium2 Bass kernel for nn_DecoderTransformer (T=2048, D=2048, H=16, V=32000).

v2 strategy (8-way tensor parallel, full inputs in / full output out):
  - All heavy matmuls run in bf16 (weights and embeddings are pre-cast to
    bf16 on the host); psum accumulation stays f32.
  - Each core gathers x = we[tok] + pe (bf16), transposes it on-chip into a
    persistent xT [D, T] SBUF tile, and computes q/k (as [hs, t]) plus v
    directly in [t, hs] layout (no later v transpose) for its 2 heads.
  - Causal attention per 512-col block with softmax-without-max; heads output
    stays in SBUF.
  - proj is sharded over the *contraction* dim (each core's 2 heads): each
    core computes a full-[D, t] partial (plus proj_b/8 so the sum carries the
    bias), and a bf16 AllReduce per t-half produces sa+pb on every core.
  - resid is built in place: xT += AR output (x was kept in xT).
  - fc is sharded over vocab (4000 cols/core); fw streams as the *moving*
    operand in natural [D, V] layout (1KB descriptors), resid chunks are the
    stationary operand, producing out[t, voc] tiles; fc_b is added via a
    broadcast tile; output is written bf16 and upcast on the host.
"""

import os

import numpy as np

T = 2048
D = 2048
H = 16
HS = 128
V = 32000
NCORES = 8
P = 128
DC = D // P            # 16 d chunks
TC = T // P            # 16 t chunks
NTB = T // 512         # 4 t-blocks of 512
HPC = H // NCORES      # 2 heads per core
VSH = V // NCORES      # 4000 vocab shard
VCW = 500              # vocab chunk width (4000 = 8*500)
NVC = VSH // VCW       # 8 vocab chunks

_CACHE = {}


def _build():
    import concourse.bass as bass
    import concourse.tile as tile
    from concourse import bacc, mybir
    from concourse.masks import make_identity

    f32 = mybir.dt.float32
    bf16 = mybir.dt.bfloat16
    i32 = mybir.dt.int32
    EXP = mybir.ActivationFunctionType.Exp
    RG = [list(range(NCORES))]

    nc = bacc.Bacc("TRN2", target_bir_lowering=False, debug=False,
                   num_devices=NCORES, num_swdge_queues=4)

    tok = nc.dram_tensor("tok", [T], i32, kind="ExternalInput")
    web = nc.dram_tensor("web", [V, D], bf16, kind="ExternalInput")
    peb = nc.dram_tensor("peb", [T, D], bf16, kind="ExternalInput")
    wqb = nc.dram_tensor("wqb", [D, HPC * HS], bf16, kind="ExternalInput")
    wkb = nc.dram_tensor("wkb", [D, HPC * HS], bf16, kind="ExternalInput")
    wvb = nc.dram_tensor("wvb", [D, HPC * HS], bf16, kind="ExternalInput")
    pwb = nc.dram_tensor("pwb", [HPC * HS, D], bf16, kind="ExternalInput")
    pb8 = nc.dram_tensor("pb8", [D, 1], f32, kind="ExternalInput")
    fwb = nc.dram_tensor("fwb", [D, VSH], bf16, kind="ExternalInput")
    fbv = nc.dram_tensor("fbv", [1, VSH], f32, kind="ExternalInput")
    out_d = nc.dram_tensor("out", [T, VSH], bf16, kind="ExternalOutput")

    with nc.allow_low_precision(reason="bf16 decoder kernel"), \
         tile.TileContext(nc) as tc:
        dram = tc.alloc_tile_pool(name="dram", bufs=1, space="DRAM")
        pconst = tc.alloc_tile_pool(name="pconst", bufs=1)
        pxT = tc.alloc_tile_pool(name="pxT", bufs=1)

        identb = pconst.tile([P, P], bf16, name="identb")
        make_identity(nc, identb[:])
        ones_col = pconst.tile([P, 1], bf16, name="ones_col")
        nc.vector.memset(ones_col[:], 1.0)
        ones_row = pconst.tile([1, P], bf16, name="ones_row")
        nc.vector.memset(ones_row[:], 1.0)
        pb8_s = pconst.tile([P, DC], f32, name="pb8_s")
        nc.sync.dma_start(out=pb8_s[:],
                          in_=pb8[:].rearrange("(dc p) one -> p (dc one)", p=P))
        fb_sb = pconst.tile([1, VSH], f32, name="fb_sb")
        nc.sync.dma_start(out=fb_sb[:], in_=fbv[:])
        fb_bf = pconst.tile([1, VSH], bf16, name="fb_bf")
        nc.vector.tensor_copy(fb_bf[:], fb_sb[:])
        # additive causal mask, shifted views: maskadd_m[s, t] with
        # m in 0..3 = maskbig[:, 384-128m : 896-128m].
        maskbig = pconst.tile([P, 896], f32, name="maskbig")
        nc.gpsimd.memset(maskbig[:], 0.0)
        nc.gpsimd.affine_select(
            out=maskbig[:], in_=maskbig[:],
            compare_op=mybir.AluOpType.is_ge, fill=-40.0,
            base=-384, pattern=[[1, 896]], channel_multiplier=-1,
        )

        xT = pxT.tile([P, DC, T], bf16, name="xT")

        partial_d = [dram.tile([D, 1024], bf16, name=f"partial{h}")
                     for h in range(2)]
        resid_sh = [dram.tile([D, 1024], bf16, name=f"resid{h}",
                              addr_space="Shared") for h in range(2)]

        with tc.tile_pool(name="pqa", bufs=1) as pqa:
            qT = pqa.tile([P, HPC, T], bf16, name="qT")
            kT = pqa.tile([P, HPC, T], bf16, name="kT")
            v_both = pqa.tile([P, TC, HPC * HS], bf16, name="v_both")
            headsT = pqa.tile([P, HPC, T], bf16, name="headsT")
            wq_s = pqa.tile([P, DC, HPC * HS], bf16, name="wq_s")
            wk_s = pqa.tile([P, DC, HPC * HS], bf16, name="wk_s")
            wv_s = pqa.tile([P, DC, HPC * HS], bf16, name="wv_s")
            pw_s = pqa.tile([P, HPC, D], bf16, name="pw_s")
            for wdram, wt in ((wqb, wq_s), (wkb, wk_s), (wvb, wv_s)):
                nc.sync.dma_start(
                    out=wt[:],
                    in_=wdram[:].rearrange("(dc p) j -> p dc j", p=P))
            nc.sync.dma_start(
                out=pw_s[:], in_=pwb[:].rearrange("(c p) d -> p c d", p=P))

            # ---- Phase 1: gather + pe add + transpose + qkv per t-block
            with tc.tile_pool(name="pemb", bufs=1) as pemb, \
                 tc.tile_pool(name="ps_tr", bufs=2, space="PSUM") as ps_tr, \
                 tc.tile_pool(name="ps_qk", bufs=4, space="PSUM") as ps_qk, \
                 tc.tile_pool(name="ps_v", bufs=2, space="PSUM") as ps_v:
                for b in range(NTB):
                    for k in range(4):
                        tcc = b * 4 + k
                        idx_t = pemb.tile([P, 1], i32, tag="idx", bufs=3,
                                          name=f"idx{tcc}")
                        nc.sync.dma_start(
                            out=idx_t[:],
                            in_=tok[:][tcc * P:(tcc + 1) * P, None])
                        xg = pemb.tile([P, D], bf16, tag="xg", bufs=4,
                                       name=f"xg{tcc}")
                        gi = nc.gpsimd.indirect_dma_start(
                            out=xg[:], out_offset=None, in_=web[:],
                            in_offset=bass.IndirectOffsetOnAxis(
                                ap=idx_t[:, :1], axis=0))
                        if tcc % 4:
                            gi.ins.queue = f"qPoolDynamic{tcc % 4}"
                        pet = pemb.tile([P, D], bf16, tag="pet", bufs=3,
                                        name=f"pet{tcc}")
                        nc.scalar.dma_start(
                            out=pet[:], in_=peb[:][tcc * P:(tcc + 1) * P, :])
                        nc.vector.tensor_add(xg[:], xg[:], pet[:])
                        for q4 in range(4):
                            tr_ps = ps_tr.tile([P, 512], bf16, tag="tr",
                                               name=f"tr{tcc}_{q4}")
                            for k4 in range(4):
                                dc = q4 * 4 + k4
                                nc.tensor.transpose(
                                    tr_ps[:, k4 * P:(k4 + 1) * P],
                                    xg[:, dc * P:(dc + 1) * P], identb[:])
                            nc.vector.tensor_copy(
                                xT[:, q4 * 4:(q4 + 1) * 4,
                                   tcc * P:(tcc + 1) * P],
                                tr_ps[:].rearrange("p (c t) -> p c t", c=4))
                    # q, k projections for this t-block ([hs, t] layout)
                    for wt, dstT in ((wq_s, qT), (wk_s, kT)):
                        for h in range(HPC):
                            ps = ps_qk.tile([P, 512], f32, tag="qkps",
                                            name=f"ps{dstT.name}{h}_{b}")
                            for dc in range(DC):
                                nc.tensor.matmul(
                                    ps[:], wt[:, dc, h * HS:(h + 1) * HS],
                                    xT[:, dc, b * 512:(b + 1) * 512],
                                    start=(dc == 0), stop=(dc == DC - 1))
                            nc.vector.tensor_copy(
                                dstT[:, h, b * 512:(b + 1) * 512], ps[:])
                    # v in [t, hs] layout (x chunks stationary, wv moving)
                    for tq in range(4):
                        tcc = b * 4 + tq
                        psv = ps_v.tile([P, HPC * HS], f32, tag="vps",
                                        name=f"psv{tcc}")
                        for dc in range(DC):
                            nc.tensor.matmul(
                                psv[:], xT[:, dc, tcc * P:(tcc + 1) * P],
                                wv_s[:, dc, :],
                                start=(dc == 0), stop=(dc == DC - 1))
                        nc.vector.tensor_copy(v_both[:, tcc, :], psv[:])

            # ---- Phase 2: causal attention + proj partials + AllReduce ----
            with tc.tile_pool(name="patt", bufs=1) as patt, \
                 tc.tile_pool(name="ps_sm", bufs=3, space="PSUM") as ps_sm, \
                 tc.tile_pool(name="ps_cs", bufs=2, space="PSUM") as ps_cs, \
                 tc.tile_pool(name="ps_av", bufs=2, space="PSUM") as ps_av, \
                 tc.tile_pool(name="ps_bc", bufs=1, space="PSUM") as ps_bc:
                for g in range(NTB):
                    for h in range(HPC):
                        nsc = 4 * g + 4
                        expT = patt.tile([P, TC, 512], bf16, tag="expT",
                                         name=f"expT{h}_{g}")
                        cs_ps = ps_cs.tile([1, 512], f32, tag="cs",
                                           name=f"cs{h}_{g}")
                        for c in range(nsc):
                            s_ps = ps_sm.tile([P, 512], f32, tag="sims",
                                              name=f"sims{h}_{g}_{c}")
                            nc.tensor.matmul(
                                s_ps[:], kT[:, h, c * P:(c + 1) * P],
                                qT[:, h, g * 512:(g + 1) * 512],
                                start=True, stop=True)
                            if c >= 4 * g:
                                m = c - 4 * g
                                nc.vector.tensor_add(
                                    s_ps[:], s_ps[:],
                                    maskbig[:, 384 - 128 * m:896 - 128 * m])
                            nc.scalar.activation(out=expT[:, c, :],
                                                 in_=s_ps[:], func=EXP)
                        for c in range(nsc):
                            nc.tensor.matmul(cs_ps[:], ones_col[:],
                                             expT[:, c, :],
                                             start=(c == 0),
                                             stop=(c == nsc - 1))
                        av_ps = ps_av.tile([P, 512], f32, tag="av",
                                           name=f"av{h}_{g}")
                        for c in range(nsc):
                            nc.tensor.matmul(
                                av_ps[:], v_both[:, c, h * HS:(h + 1) * HS],
                                expT[:, c, :],
                                start=(c == 0), stop=(c == nsc - 1))
                        recip = patt.tile([1, 512], f32, tag="recip",
                                          bufs=2, name=f"rc{h}_{g}")
                        nc.vector.reciprocal(recip[:], cs_ps[:])
                        recb = patt.tile([1, 512], bf16, tag="recb",
                                         bufs=2, name=f"rcb{h}_{g}")
                        nc.vector.tensor_copy(recb[:], recip[:])
                        bc_ps = ps_bc.tile([P, 512], f32, tag="bc",
                                           name=f"bc{h}_{g}")
                        nc.tensor.matmul(bc_ps[:], ones_row[:], recb[:],
                                         start=True, stop=True)
                        bc_s = patt.tile([P, 512], f32, tag="bc_s",
                                         bufs=2, name=f"bcs{h}_{g}")
                        nc.vector.tensor_copy(bc_s[:], bc_ps[:])
                        nc.vector.tensor_mul(
                            headsT[:, h, g * 512:(g + 1) * 512],
                            av_ps[:], bc_s[:])
                    if g % 2 == 1:
                        half = g // 2
                        # proj partial for this t-half (contraction over the
                        # core's 2 heads), +proj_b/8 so the AllReduce sums to
                        # sa + proj_b on every core.
                        for tb2 in range(2):
                            psb = patt.tile([P, DC, 512], bf16, tag="psb",
                                            name=f"psb{half}_{tb2}")
                            for do in range(DC):
                                pp = ps_sm.tile([P, 512], f32, tag="sims",
                                                name=f"pp{half}{tb2}_{do}")
                                for c in range(HPC):
                                    nc.tensor.matmul(
                                        pp[:], pw_s[:, c, do * P:(do + 1) * P],
                                        headsT[:, c, half * 1024 + tb2 * 512:
                                               half * 1024 + (tb2 + 1) * 512],
                                        start=(c == 0), stop=(c == HPC - 1))
                                nc.vector.tensor_scalar_add(
                                    psb[:, do, :], pp[:], pb8_s[:, do:do + 1])
                            nc.scalar.dma_start(
                                out=partial_d[half][:, tb2 * 512:
                                                    (tb2 + 1) * 512]
                                .rearrange("(dc p) t -> p dc t", p=P),
                                in_=psb[:])
                        nc.gpsimd.collective_compute(
                            "AllReduce", mybir.AluOpType.add,
                            replica_groups=RG,
                            ins=[partial_d[half][:]],
                            outs=[resid_sh[half][:]])

        # ---- Phase 3: resid = x + AR(sa+pb) in place; fc over vocab ----
        with tc.tile_pool(name="pfc", bufs=1) as pfc, \
             tc.tile_pool(name="ps_fc", bufs=6, space="PSUM") as ps_fc:
            fb_bcast = pfc.tile([P, VSH], f32, name="fb_bcast")
            for vc in range(NVC):
                fps = ps_fc.tile([P, VCW], f32, tag="fcps", name=f"fbb{vc}")
                nc.tensor.matmul(fps[:], ones_row[:],
                                 fb_bf[:, vc * VCW:(vc + 1) * VCW],
                                 start=True, stop=True)
                nc.vector.tensor_copy(fb_bcast[:, vc * VCW:(vc + 1) * VCW],
                                      fps[:])

            def build_resid(half):
                ast = pfc.tile([P, DC, 1024], bf16, tag="arst",
                               name=f"arst{half}")
                nc.sync.dma_start(
                    out=ast[:],
                    in_=resid_sh[half][:].rearrange("(dc p) t -> p dc t", p=P))
                nc.vector.tensor_add(
                    xT[:, :, half * 1024:(half + 1) * 1024],
                    xT[:, :, half * 1024:(half + 1) * 1024], ast[:])

            def load_fw(vc, nm):
                t = pfc.tile([P, DC, VCW], bf16, tag="fw_t", bufs=3, name=nm)
                gi = nc.gpsimd.dma_start(
                    out=t[:],
                    in_=fwb[:][:, vc * VCW:(vc + 1) * VCW]
                    .rearrange("(dc p) v -> p dc v", p=P))
                if vc % 4:
                    gi.ins.queue = f"qPoolDynamic{vc % 4}"
                return t

            def fc_pass(vc, half, fw_t, tag):
                for tc8 in range(8):
                    toff = half * 1024 + tc8 * P
                    ps = ps_fc.tile([P, VCW], f32, tag="fcps",
                                    name=f"fc{tag}_{vc}_{half}_{tc8}")
                    for dc in range(DC):
                        nc.tensor.matmul(
                            ps[:], xT[:, dc, toff:toff + P],
                            fw_t[:, dc, :],
                            start=(dc == 0), stop=(dc == DC - 1))
                    ev = pfc.tile([P, VCW], bf16, tag="fc_ev", bufs=4,
                                  name=f"fcev{tag}_{vc}_{half}_{tc8}")
                    nc.vector.tensor_add(
                        ev[:], ps[:], fb_bcast[:, vc * VCW:(vc + 1) * VCW])
                    nc.scalar.dma_start(
                        out=out_d[:][toff:toff + P,
                                     vc * VCW:(vc + 1) * VCW],
                        in_=ev[:])

            # vc 0 runs t-half 0 first (only AR0 needed), halves interleave
            # afterwards; vc 0's t-half 1 runs last with a small fw reload.
            fw0 = load_fw(0, "fw0a")
            build_resid(0)
            fc_pass(0, 0, fw0, "a")
            build_resid(1)
            for vc in range(1, NVC):
                fwt = load_fw(vc, f"fw{vc}")
                fc_pass(vc, 0, fwt, "m")
                fc_pass(vc, 1, fwt, "m")
            fw0b = load_fw(0, "fw0b")
            fc_pass(0, 1, fw0b, "b")

        pxT.release()
        pconst.release()
        dram.release()

    if not int(os.environ.get("BASSKERNEL_SKIP_COMPILE", "0")):
        nc.compile()
    return nc


def _get_nc():
    if "nc" not in _CACHE:
        _CACHE["nc"] = _build()
    return _CACHE["nc"]


def kernel(token_ids, we, pe, Wq, Wk, Wv, proj_w, proj_b, fc_w, fc_b):
    import ml_dtypes

    from concourse.bass_utils import run_bass_kernel_spmd

    bf16 = ml_dtypes.bfloat16

    tok = np.asarray(token_ids).astype(np.int32)
    web = np.ascontiguousarray(np.asarray(we)).astype(bf16)
    peb = np.ascontiguousarray(np.asarray(pe))[:T].astype(bf16)
    Wq = np.asarray(Wq, dtype=np.float32)
    Wk = np.asarray(Wk, dtype=np.float32)
    Wv = np.asarray(Wv, dtype=np.float32)
    proj_w = np.asarray(proj_w, dtype=np.float32)
    proj_b = np.asarray(proj_b, dtype=np.float32)
    fc_w = np.asarray(fc_w, dtype=np.float32)
    fc_b = np.asarray(fc_b, dtype=np.float32)

    scale = np.float32(1.0 / np.sqrt(HS))
    pb8 = (proj_b / NCORES).reshape(D, 1).astype(np.float32)
    in_maps = []
    for i in range(NCORES):
        h0 = HPC * i
        wq_i = np.ascontiguousarray(
            np.concatenate([Wq[h0 + j] for j in range(HPC)], axis=1)) * scale
        wk_i = np.ascontiguousarray(
            np.concatenate([Wk[h0 + j] for j in range(HPC)], axis=1))
        wv_i = np.ascontiguousarray(
            np.concatenate([Wv[h0 + j] for j in range(HPC)], axis=1))
        pw_i = np.ascontiguousarray(
            proj_w[HPC * HS * i:HPC * HS * (i + 1), :])
        fw_i = np.ascontiguousarray(fc_w[:, VSH * i:VSH * (i + 1)])
        fb_i = np.ascontiguousarray(
            fc_b[VSH * i:VSH * (i + 1)].reshape(1, VSH)).astype(np.float32)
        in_maps.append({
            "tok": tok, "web": web, "peb": peb,
            "wqb": wq_i.astype(bf16), "wkb": wk_i.astype(bf16),
            "wvb": wv_i.astype(bf16),
            "pwb": pw_i.astype(bf16), "pb8": pb8,
            "fwb": fw_i.astype(bf16), "fbv": fb_i,
        })

    nc = _get_nc()
    trace = bool(int(os.environ.get("BASSKERNEL_TRACE", "0")))
    res = run_bass_kernel_spmd(nc, in_maps, core_ids=list(range(NCORES)),
                               trace=trace)
    if trace and res.exec_time_ns is not None:
        print(f"HW exec time: {res.exec_time_ns} ns")
        if res.instructions_and_trace is not None:
            print(f"Trace: {res.instructions_and_trace[1]}")

    out = np.empty((T, V), dtype=np.float32)
    for i in range(NCORES):
        out[:, VSH * i:VSH * (i + 1)] = res.results[i]["out"].astype(
            np.float32)
    return out


# revision 9
# speedup vs baseline: 1.3208x; 1.0154x over previous
"""Trainium2 Bass kernel for nn_DecoderTransformer (T=2048, D=2048, H=16, V=32000).

v3 strategy (8-way tensor parallel, full inputs in / full output out):
  - All matmuls bf16 (inputs pre-cast on host), f32 psum accumulation.
  - Fully interleaved prefix: per 512-token block b, gather x = we[tok]+pe
    (bf16), XBAR DMA-transpose it into a persistent xT [D, T] tile (no PE
    transposes at all), project q/k ([hs, t]) and v ([t, hs]) for the core's
    2 heads, run causal attention for block b, compute the proj partial for
    those 512 columns (contraction over the 2 local heads, +proj_b/8), and
    issue a bf16 AllReduce per quarter.  AR(q0) completes while later blocks
    still compute.
  - resid built in place per quarter: xT[:, q] += AR output.
  - fc sharded over vocab (4000 cols/core), fw streamed as the moving operand
    in natural layout; out[t, voc] written bf16 and upcast on the host.
    Half-major (t 0:1024 then 1024:2048) so only AR0/AR1 gate the start.
"""

import os

import numpy as np

T = 2048
D = 2048
H = 16
HS = 128
V = 32000
NCORES = 8
P = 128
DC = D // P            # 16 d chunks
TC = T // P            # 16 t chunks
NTB = T // 512         # 4 t-blocks of 512
HPC = H // NCORES      # 2 heads per core
VSH = V // NCORES      # 4000 vocab shard
VCW = 500              # vocab chunk width (4000 = 8*500)
NVC = VSH // VCW       # 8 vocab chunks

_CACHE = {}


def _build():
    import concourse.bass as bass
    import concourse.tile as tile
    from concourse import bacc, mybir

    f32 = mybir.dt.float32
    bf16 = mybir.dt.bfloat16
    i32 = mybir.dt.int32
    EXP = mybir.ActivationFunctionType.Exp
    IDENT = mybir.ActivationFunctionType.Identity
    RG = [list(range(NCORES))]

    nc = bacc.Bacc("TRN2", target_bir_lowering=False, debug=False,
                   num_devices=NCORES, num_swdge_queues=4)

    tok = nc.dram_tensor("tok", [T], i32, kind="ExternalInput")
    web = nc.dram_tensor("web", [V, D], bf16, kind="ExternalInput")
    peb = nc.dram_tensor("peb", [T, D], bf16, kind="ExternalInput")
    wqb = nc.dram_tensor("wqb", [D, HPC * HS], bf16, kind="ExternalInput")
    wkb = nc.dram_tensor("wkb", [D, HPC * HS], bf16, kind="ExternalInput")
    wvb = nc.dram_tensor("wvb", [D, HPC * HS], bf16, kind="ExternalInput")
    pwb = nc.dram_tensor("pwb", [HPC * HS, D], bf16, kind="ExternalInput")
    pb8 = nc.dram_tensor("pb8", [D, 1], f32, kind="ExternalInput")
    fwb = nc.dram_tensor("fwb", [D, VSH], bf16, kind="ExternalInput")
    fbv = nc.dram_tensor("fbv", [1, VSH], f32, kind="ExternalInput")
    out_d = nc.dram_tensor("out", [T, VSH], bf16, kind="ExternalOutput")

    with nc.allow_low_precision(reason="bf16 decoder kernel"), \
         tile.TileContext(nc) as tc:
        dram = tc.alloc_tile_pool(name="dram", bufs=1, space="DRAM")
        pconst = tc.alloc_tile_pool(name="pconst", bufs=1)
        pxT = tc.alloc_tile_pool(name="pxT", bufs=1)

        ones_col = pconst.tile([P, 1], bf16, name="ones_col")
        nc.vector.memset(ones_col[:], 1.0)
        ones_row = pconst.tile([1, P], bf16, name="ones_row")
        nc.vector.memset(ones_row[:], 1.0)
        pb8_s = pconst.tile([P, DC], f32, name="pb8_s")
        nc.sync.dma_start(out=pb8_s[:],
                          in_=pb8[:].rearrange("(dc p) one -> p (dc one)", p=P))
        # additive causal mask, shifted views: maskadd_m[s, t] with
        # m in 0..3 = maskbig[:, 384-128m : 896-128m].
        maskbig = pconst.tile([P, 896], f32, name="maskbig")
        nc.gpsimd.memset(maskbig[:], 0.0)
        nc.gpsimd.affine_select(
            out=maskbig[:], in_=maskbig[:],
            compare_op=mybir.AluOpType.is_ge, fill=-40.0,
            base=-384, pattern=[[1, 896]], channel_multiplier=-1,
        )

        xT = pxT.tile([P, DC, T], bf16, name="xT")

        partial_d = [dram.tile([D, 512], bf16, name=f"partial{q}")
                     for q in range(NTB)]
        resid_q = [dram.tile([D, 512], bf16, name=f"resid{q}",
                             addr_space="Shared") for q in range(NTB)]

        with tc.tile_pool(name="pqa", bufs=1) as pqa, \
             tc.tile_pool(name="pemb", bufs=1) as pemb, \
             tc.tile_pool(name="patt", bufs=1) as patt, \
             tc.tile_pool(name="ps_qk", bufs=2, space="PSUM") as ps_qk, \
             tc.tile_pool(name="ps_v", bufs=1, space="PSUM") as ps_v, \
             tc.tile_pool(name="ps_sm", bufs=2, space="PSUM") as ps_sm, \
             tc.tile_pool(name="ps_cs", bufs=1, space="PSUM") as ps_cs, \
             tc.tile_pool(name="ps_av", bufs=1, space="PSUM") as ps_av, \
             tc.tile_pool(name="ps_bc", bufs=1, space="PSUM") as ps_bc:
            kT = pqa.tile([P, HPC, T], bf16, name="kT")
            v_both = pqa.tile([P, TC, HPC * HS], bf16, name="v_both")
            wq_s = pqa.tile([P, DC, HPC * HS], bf16, name="wq_s")
            wk_s = pqa.tile([P, DC, HPC * HS], bf16, name="wk_s")
            wv_s = pqa.tile([P, DC, HPC * HS], bf16, name="wv_s")
            pw_s = pqa.tile([P, HPC, D], bf16, name="pw_s")
            for wdram, wt in ((wqb, wq_s), (wkb, wk_s), (wvb, wv_s)):
                nc.sync.dma_start(
                    out=wt[:],
                    in_=wdram[:].rearrange("(dc p) j -> p dc j", p=P))
            nc.sync.dma_start(
                out=pw_s[:], in_=pwb[:].rearrange("(c p) d -> p c d", p=P))

            for b in range(NTB):
                # ---- gather + pe add + XBAR transpose, 4 t-chunks ----
                for k in range(4):
                    tcc = b * 4 + k
                    idx_t = pemb.tile([P, 1], i32, tag="idx", bufs=3,
                                      name=f"idx{tcc}")
                    nc.sync.dma_start(
                        out=idx_t[:],
                        in_=tok[:][tcc * P:(tcc + 1) * P, None])
                    xg = pemb.tile([P, D], bf16, tag="xg", bufs=4,
                                   name=f"xg{tcc}")
                    gi = nc.gpsimd.indirect_dma_start(
                        out=xg[:], out_offset=None, in_=web[:],
                        in_offset=bass.IndirectOffsetOnAxis(
                            ap=idx_t[:, :1], axis=0))
                    if tcc % 4:
                        gi.ins.queue = f"qPoolDynamic{tcc % 4}"
                    pet = pemb.tile([P, D], bf16, tag="pet", bufs=3,
                                    name=f"pet{tcc}")
                    nc.scalar.dma_start(
                        out=pet[:], in_=peb[:][tcc * P:(tcc + 1) * P, :])
                    nc.vector.tensor_add(xg[:], xg[:], pet[:])
                    nc.sync.dma_start_transpose(
                        out=xT[:, :, tcc * P:(tcc + 1) * P], in_=xg[:])
                # ---- q, k for block b ([hs, t]); v in [t, hs] ----
                qTb = patt.tile([P, HPC, 512], bf16, tag="qTb", bufs=2,
                                name=f"qTb{b}")
                for wt, dst in ((wq_s, qTb), (wk_s, None)):
                    for h in range(HPC):
                        ps = ps_qk.tile([P, 512], f32, tag="qkps",
                                        name=f"qk{wt.name}{h}_{b}")
                        for dc in range(DC):
                            nc.tensor.matmul(
                                ps[:], wt[:, dc, h * HS:(h + 1) * HS],
                                xT[:, dc, b * 512:(b + 1) * 512],
                                start=(dc == 0), stop=(dc == DC - 1))
                        if dst is None:
                            nc.vector.tensor_copy(
                                kT[:, h, b * 512:(b + 1) * 512], ps[:])
                        else:
                            nc.vector.tensor_copy(dst[:, h, :], ps[:])
                for tq in range(4):
                    tcc = b * 4 + tq
                    psv = ps_v.tile([P, HPC * HS], f32, tag="vps",
                                    name=f"psv{tcc}")
                    for dc in range(DC):
                        nc.tensor.matmul(
                            psv[:], xT[:, dc, tcc * P:(tcc + 1) * P],
                            wv_s[:, dc, :],
                            start=(dc == 0), stop=(dc == DC - 1))
                    nc.vector.tensor_copy(v_both[:, tcc, :], psv[:])

                # ---- causal attention for block g = b ----
                g = b
                headsb = patt.tile([P, HPC, 512], bf16, tag="headsb", bufs=2,
                                   name=f"headsb{g}")
                for h in range(HPC):
                    nsc = 4 * g + 4
                    expT = patt.tile([P, TC, 512], bf16, tag="expT",
                                     name=f"expT{h}_{g}")
                    cs_ps = ps_cs.tile([1, 512], f32, tag="cs",
                                       name=f"cs{h}_{g}")
                    for c in range(nsc):
                        s_ps = ps_sm.tile([P, 512], f32, tag="sims",
                                          name=f"sims{h}_{g}_{c}")
                        nc.tensor.matmul(
                            s_ps[:], kT[:, h, c * P:(c + 1) * P],
                            qTb[:, h, :], start=True, stop=True)
                        if c >= 4 * g:
                            m = c - 4 * g
                            nc.vector.tensor_add(
                                s_ps[:], s_ps[:],
                                maskbig[:, 384 - 128 * m:896 - 128 * m])
                        nc.scalar.activation(out=expT[:, c, :],
                                             in_=s_ps[:], func=EXP)
                    for c in range(nsc):
                        nc.tensor.matmul(cs_ps[:], ones_col[:],
                                         expT[:, c, :],
                                         start=(c == 0), stop=(c == nsc - 1))
                    av_ps = ps_av.tile([P, 512], f32, tag="av",
                                       name=f"av{h}_{g}")
                    for c in range(nsc):
                        nc.tensor.matmul(
                            av_ps[:], v_both[:, c, h * HS:(h + 1) * HS],
                            expT[:, c, :],
                            start=(c == 0), stop=(c == nsc - 1))
                    recip = patt.tile([1, 512], f32, tag="recip",
                                      bufs=2, name=f"rc{h}_{g}")
                    nc.vector.reciprocal(recip[:], cs_ps[:])
                    recb = patt.tile([1, 512], bf16, tag="recb",
                                     bufs=2, name=f"rcb{h}_{g}")
                    nc.vector.tensor_copy(recb[:], recip[:])
                    bc_ps = ps_bc.tile([P, 512], f32, tag="bc",
                                       name=f"bc{h}_{g}")
                    nc.tensor.matmul(bc_ps[:], ones_row[:], recb[:],
                                     start=True, stop=True)
                    bc_s = patt.tile([P, 512], f32, tag="bc_s",
                                     bufs=2, name=f"bcs{h}_{g}")
                    nc.vector.tensor_copy(bc_s[:], bc_ps[:])
                    nc.vector.tensor_mul(headsb[:, h, :], av_ps[:], bc_s[:])

                # ---- proj partial for quarter b (+pb/8), then AllReduce ----
                psb = patt.tile([P, DC, 512], bf16, tag="psb",
                                name=f"psb{b}")
                for do in range(DC):
                    pp = ps_sm.tile([P, 512], f32, tag="sims",
                                    name=f"pp{b}_{do}")
                    for c in range(HPC):
                        nc.tensor.matmul(
                            pp[:], pw_s[:, c, do * P:(do + 1) * P],
                            headsb[:, c, :],
                            start=(c == 0), stop=(c == HPC - 1))
                    nc.scalar.activation(out=psb[:, do, :], in_=pp[:],
                                         func=IDENT,
                                         bias=pb8_s[:, do:do + 1])
                nc.scalar.dma_start(
                    out=partial_d[b][:].rearrange("(dc p) t -> p dc t", p=P),
                    in_=psb[:])
                nc.gpsimd.collective_compute(
                    "AllReduce", mybir.AluOpType.add, replica_groups=RG,
                    ins=[partial_d[b][:]], outs=[resid_q[b][:]])

        # ---- fc phase: resid in place, then out = resid @ fw + fb ----
        with tc.tile_pool(name="pfc", bufs=1) as pfc, \
             tc.tile_pool(name="ps_fc", bufs=6, space="PSUM") as ps_fc:
            fb_sb = pfc.tile([1, VSH], f32, name="fb_sb")
            nc.scalar.dma_start(out=fb_sb[:], in_=fbv[:])
            fb_bf = pfc.tile([1, VSH], bf16, name="fb_bf")
            nc.vector.tensor_copy(fb_bf[:], fb_sb[:])
            fb_bcast = pfc.tile([P, VSH], f32, name="fb_bcast")
            for vc in range(NVC):
                fps = ps_fc.tile([P, VCW], f32, tag="fcps", name=f"fbb{vc}")
                nc.tensor.matmul(fps[:], ones_row[:],
                                 fb_bf[:, vc * VCW:(vc + 1) * VCW],
                                 start=True, stop=True)
                nc.vector.tensor_copy(fb_bcast[:, vc * VCW:(vc + 1) * VCW],
                                      fps[:])

            def build_resid(q, eng):
                ast = pfc.tile([P, DC, 512], bf16, tag="arst", bufs=2,
                               name=f"arst{q}")
                eng.dma_start(
                    out=ast[:],
                    in_=resid_q[q][:].rearrange("(dc p) t -> p dc t", p=P))
                nc.vector.tensor_add(
                    xT[:, :, q * 512:(q + 1) * 512],
                    xT[:, :, q * 512:(q + 1) * 512], ast[:])

            def load_fw(vc, nm, eng):
                t = pfc.tile([P, DC, VCW], bf16, tag="fw_t", bufs=3, name=nm)
                eng.dma_start(
                    out=t[:],
                    in_=fwb[:][:, vc * VCW:(vc + 1) * VCW]
                    .rearrange("(dc p) v -> p dc v", p=P))
                return t

            def fc_pass(vc, half, fw_t, tag):
                for tc8 in range(8):
                    toff = half * 1024 + tc8 * P
                    ps = ps_fc.tile([P, VCW], f32, tag="fcps",
                                    name=f"fc{tag}_{vc}_{half}_{tc8}")
                    for dc in range(DC):
                        nc.tensor.matmul(
                            ps[:], xT[:, dc, toff:toff + P],
                            fw_t[:, dc, :],
                            start=(dc == 0), stop=(dc == DC - 1))
                    ev = pfc.tile([P, VCW], bf16, tag="fc_ev", bufs=4,
                                  name=f"fcev{tag}_{vc}_{half}_{tc8}")
                    nc.vector.tensor_add(
                        ev[:], ps[:], fb_bcast[:, vc * VCW:(vc + 1) * VCW])
                    nc.scalar.dma_start(
                        out=out_d[:][toff:toff + P,
                                     vc * VCW:(vc + 1) * VCW],
                        in_=ev[:])

            # sync stream: fw0, arst0, fw1, arst1, fw2.. ; gpsimd: arst2/3
            fw0 = load_fw(0, "fw0a", nc.sync)
            build_resid(0, nc.sync)
            fw1 = load_fw(1, "fw1a", nc.sync)
            build_resid(1, nc.sync)
            fwt = {0: fw0, 1: fw1}
            for vc in range(2, NVC):
                fwt[vc] = load_fw(vc, f"fw{vc}a", nc.sync)
            build_resid(2, nc.gpsimd)
            build_resid(3, nc.gpsimd)
            for vc in range(NVC):
                fc_pass(vc, 0, fwt[vc], "a")
            for vc in range(NVC):
                fc_pass(vc, 1, load_fw(vc, f"fw{vc}b",
                                       nc.gpsimd if vc % 2 else nc.sync), "b")

        pxT.release()
        pconst.release()
        dram.release()

    if not int(os.environ.get("BASSKERNEL_SKIP_COMPILE", "0")):
        nc.compile()
    return nc


def _get_nc():
    if "nc" not in _CACHE:
        _CACHE["nc"] = _build()
    return _CACHE["nc"]


def kernel(token_ids, we, pe, Wq, Wk, Wv, proj_w, proj_b, fc_w, fc_b):
    import ml_dtypes

    from concourse.bass_utils import run_bass_kernel_spmd

    bf16 = ml_dtypes.bfloat16

    tok = np.asarray(token_ids).astype(np.int32)
    web = np.ascontiguousarray(np.asarray(we)).astype(bf16)
    peb = np.ascontiguousarray(np.asarray(pe))[:T].astype(bf16)
    Wq = np.asarray(Wq, dtype=np.float32)
    Wk = np.asarray(Wk, dtype=np.float32)
    Wv = np.asarray(Wv, dtype=np.float32)
    proj_w = np.asarray(proj_w, dtype=np.float32)
    proj_b = np.asarray(proj_b, dtype=np.float32)
    fc_w = np.asarray(fc_w, dtype=np.float32)
    fc_b = np.asarray(fc_b, dtype=np.float32)

    scale = np.float32(1.0 / np.sqrt(HS))
    pb8 = (proj_b / NCORES).reshape(D, 1).astype(np.float32)
    in_maps = []
    for i in range(NCORES):
        h0 = HPC * i
        wq_i = np.ascontiguousarray(
            np.concatenate([Wq[h0 + j] for j in range(HPC)], axis=1)) * scale
        wk_i = np.ascontiguousarray(
            np.concatenate([Wk[h0 + j] for j in range(HPC)], axis=1))
        wv_i = np.ascontiguousarray(
            np.concatenate([Wv[h0 + j] for j in range(HPC)], axis=1))
        pw_i = np.ascontiguousarray(
            proj_w[HPC * HS * i:HPC * HS * (i + 1), :])
        fw_i = np.ascontiguousarray(fc_w[:, VSH * i:VSH * (i + 1)])
        fb_i = np.ascontiguousarray(
            fc_b[VSH * i:VSH * (i + 1)].reshape(1, VSH)).astype(np.float32)
        in_maps.append({
            "tok": tok, "web": web, "peb": peb,
            "wqb": wq_i.astype(bf16), "wkb": wk_i.astype(bf16),
            "wvb": wv_i.astype(bf16),
            "pwb": pw_i.astype(bf16), "pb8": pb8,
            "fwb": fw_i.astype(bf16), "fbv": fb_i,
        })

    nc = _get_nc()
    trace = bool(int(os.environ.get("BASSKERNEL_TRACE", "0")))
    res = run_bass_kernel_spmd(nc, in_maps, core_ids=list(range(NCORES)),
                               trace=trace)
    if trace and res.exec_time_ns is not None:
        print(f"HW exec time: {res.exec_time_ns} ns")
        if res.instructions_and_trace is not None:
            print(f"Trace: {res.instructions_and_trace[1]}")

    out = np.empty((T, V), dtype=np.float32)
    for i in range(NCORES):
        out[:, VSH * i:VSH * (i + 1)] = res.results[i]["out"].astype(
            np.float32)
    return out
